# revision 54
# baseline (speedup 1.0000x reference)
"""Trainium2 Bass kernel for nn_AttentionBlock (GroupNorm -> MHA(8 heads,
s=4096) -> proj -> residual).

Sharding: 8 cores = 2 batches x 4 query-token slices (1024 tokens each).
Each core: GroupNorm stats + full K/V for its batch, Q for its slice,
streaming softmax attention over its slice, projection + residual.

Design notes (vs the earlier bf16 baseline):
- x uploads as fp8e4 (1MB) plus an f32 residual slice; GroupNorm's
  per-channel affine (A, B) is folded into the QKV weights at runtime, so xn
  is never materialized (saves a full DVE pass).  QKV weights upload bf16 and
  are runtime-scaled by A into fp8.
- K/Q/V/proj matmuls are plain fp8 2-pass accumulate (measured faster than
  fp8 DoubleRow here: DR's ldweights penalty eats the pair win at these
  free dims).  Scores are bf16 with 4-way PE row packing; AV is bf16
  33-wide (V + ones denominator column) with 2-way PE column packing.
- exp split: ACT hardware Exp for the (h0,h0+1) tiles; DVE 3-uop cubic
  custom op (1 + C1 s + s^2 + C0 s^3 ~ exp(s/g), scores prescaled by
  g=1/sqrt(2) in the host weight scale) for all (h0+2,h0+3) tiles.
  Rates: ACT ~1.11us, DVE ~1.22us per [128,1024] tile (DVE is 1x from
  f32 PSUM; TRN2 matmul cannot write 16-bit PSUM, so no 2X_1PORT).
- softmax denominator rides the AV accumulation via the ones column.
  rec = exp(-ln(den) + ln(ATT_UP)): Ln on the den rows, tiny PE broadcast
  matmuls, then ACT Exp.  Together with GN's rsqrt = exp(-0.5 ln(var+eps))
  every ACT function used lives in the natural_log_exp_and_others table
  set (enforced by trimming Exp/Ln from the other sets pre-compile), so
  exactly one ACT_TABLE_LOAD is emitted (vs ~10 reloads at 2.7us each
  when Reciprocal/Sqrt alternate with Exp).
- AV evacuation is fused with normalization: attn8 = av(psum) * rec in 4
  [32,512] DVE tensor_tensors (no out_un8 staging).
- dtype scale management: wq/wk host-scaled by 64, wv/pw by 16 to keep fp8
  weights in the e4m3 normal range; compensated at psum evacuation.
  attn8 is upscaled by 64 (folded into the denominator) for the same reason.

Scheduling notes (measured, not guessed):
- steady state is three-way engine-balanced: PE ~87%, ACT ~82%, DVE ~78%;
  per chunk 2 exp tiles (ACT+DVE in parallel) bound the cadence at
  ~1.2us with PE close behind.
- interleaving head-phase K/Q/V pieces or deferred softmax tails into the
  attention chunk loop via the ps_sc psum ring REGRESSES (ring-depth
  collapse serializes the score pipeline); keep the head serial and the
  tails inline.
- PSUM is fully committed: 3x [128,1024] f32 score tiles + 2 AV banks;
  rec_ps reuses a ps_sc slot at the tail.

Self-contained: hardcodes shapes (x: (2,256,64,64) f32).
"""

import math
import sys

import numpy as np

sys.path.insert(0, "/opt/trn_rl_repo")

import ml_dtypes  # noqa: E402

F8 = ml_dtypes.float8_e4m3
BF16 = ml_dtypes.bfloat16

# ---- problem constants ----
B, C, H, W = 2, 256, 64, 64
S = H * W            # 4096 tokens
NH, HD = 8, 32       # heads, head dim
GROUPS = 32
CPG = C // GROUPS
EPS = 1e-5
NCORES = 8
NSL = 4              # token slices per batch
SL = S // NSL        # 1024 query tokens per core
NT = C // 128        # 2 channel tiles
TCH = S // 128       # 32 key chunks
PAIRS = TCH // 2     # 16 key chunk pairs

# score prescale gamma: psum scores = GAMMA * s_true; exp(s') with the DVE
# cubic 1 + C1 s' + s'^2 + C0 s'^3 needs quad coeff 1 -> GAMMA = 1/sqrt(2)
GAMMA = 2.0 ** -0.5
C1_DVE = 1.414203
C0_DVE = 0.473414
WQK_HOST = 64.0      # fp8 range scale on wq/wk (compensated at K/Q evac)
WVP_HOST = 16.0      # fp8 range scale on wv/pw (compensated at V evac / den)
ATT_UP = 64.0        # attn8 upscale (folded into den) to stay in fp8 normal
LA = 2               # exp->AV software pipeline lookahead (chunks)

_PROGRAM = None
LAST_RESULTS = None
_EXPC_OP = None


def _register_expc():
    """ANT_EXPC3: 1 + s*(C1 + s*(1 + s*C0)) = 1 + C1 s + s^2 + C0 s^3."""
    global _EXPC_OP
    if _EXPC_OP is not None:
        return _EXPC_OP
    from concourse import dve_ops
    from concourse.dve_spec import Spec, Src0, C0, C1, One, lower
    from concourse.dve_uop import DveOpSpec
    from concourse.dve_ops import DveOp

    for op in dve_ops.OPS:
        if op.name == "ANT_EXPC3":
            _EXPC_OP = op
            return op

    def _ref(in0, in1, s0, s1, imm2):
        t1 = 1.0 + in0 * s0
        t2 = s1 + in0 * t1
        t3 = 1.0 + in0 * t2
        return t3.astype(np.float32)

    t1 = One + Src0 * C0
    t2 = C1 + Src0 * t1
    t3 = One + Src0 * t2
    spec = Spec(body=t3, reference=_ref)
    shas = {
        ver: DveOpSpec(
            name="ANT_EXPC3", opcode=0, uops=lower(spec, ver=ver), rd1_en=False
        ).sha(ver)
        for ver in ("v3", "v4")
    }
    op = DveOp("ANT_EXPC3", spec, subdim=False, uops_sha=shas)
    dve_ops.OPS.append(op)
    dve_ops._SUB_OPCODE_FOR_NAME[op.name] = (
        dve_ops._CUSTOM_DVE_ROW_BASE + len(dve_ops.OPS) - 1
    )
    dve_ops.CUSTOM_DVE_SPECS[op.name] = spec
    _EXPC_OP = op
    return op


def _build_program():
    import concourse.bass as bass  # noqa: F401
    import concourse.tile as tile
    from concourse import bacc, mybir

    f32 = mybir.dt.float32
    bf16 = mybir.dt.bfloat16
    f8 = mybir.dt.float8e4
    Alu = mybir.AluOpType
    Act = mybir.ActivationFunctionType
    DR = mybir.MatmulPerfMode.DoubleRow
    DP = mybir.MatmulPerfMode.DoublePixel

    expc = _register_expc()

    # All ACT functions this kernel uses (Exp, Ln, Copy, Identity) live in
    # the natural_log_exp_and_others table set.  The load-insertion pass
    # would otherwise pick exp_and_others for Exp and natural_log for Ln,
    # reloading ACT tables (~2.7us each) at every softmax tail.  Trim Exp/Ln
    # from the other sets (membership only -- set order/ids stay valid) so
    # one set covers everything and exactly one load is emitted.
    from concourse import hw_specs

    nc = bacc.Bacc(
        "TRN2",
        target_bir_lowering=False,
        debug=False,
        enable_asserts=False,
        num_devices=NCORES,
    )

    _keep = "natural_log_exp_and_others"
    _tabs = hw_specs.get_activation_tables(nc.m.arch)
    assert _keep in _tabs and Act.Exp in _tabs[_keep] and Act.Ln in _tabs[_keep]
    for _name, _fns in _tabs.items():
        if _name != _keep:
            _fns.discard(Act.Exp)
            _fns.discard(Act.Ln)

    # ---- DRAM I/O ----
    x8_d = nc.dram_tensor("x8", [C, S], f8, kind="ExternalInput").ap()
    x8sl_d = nc.dram_tensor("x8sl", [C, SL], f8, kind="ExternalInput").ap()
    xsl_d = nc.dram_tensor("xsl", [C, SL], f32, kind="ExternalInput").ap()
    wq_d = nc.dram_tensor("wq_t", [C, C], bf16, kind="ExternalInput").ap()
    wk_d = nc.dram_tensor("wk_t", [C, C], bf16, kind="ExternalInput").ap()
    wv_d = nc.dram_tensor("wv_t", [C, C], bf16, kind="ExternalInput").ap()
    pw8_d = nc.dram_tensor("pw8_t", [C, C], f8, kind="ExternalInput").ap()
    bq_d = nc.dram_tensor("bq", [C, 1], f32, kind="ExternalInput").ap()
    bk_d = nc.dram_tensor("bk", [C, 1], f32, kind="ExternalInput").ap()
    pb_d = nc.dram_tensor("pb", [C, 1], f32, kind="ExternalInput").ap()
    gnw_d = nc.dram_tensor("gnw", [C, 1], f32, kind="ExternalInput").ap()
    gnb_d = nc.dram_tensor("gnb", [C, 1], f32, kind="ExternalInput").ap()
    g8_d = nc.dram_tensor("g8", [128, 16], f32, kind="ExternalInput").ap()
    g8t_d = nc.dram_tensor("g8t", [16, 128], f32, kind="ExternalInput").ap()
    out_d = nc.dram_tensor("out_sl", [C, SL], f32, kind="ExternalOutput").ap()

    with tile.TileContext(nc) as tc:
        with tc.tile_pool(name="consts", bufs=1) as consts, \
             tc.tile_pool(name="data", bufs=1) as data, \
             tc.tile_pool(name="work", bufs=3) as work:

            # ---------- load inputs ----------
            x8_sb = data.tile([128, NT, S], f8)
            for t in range(NT):
                for hh in range(2):
                    sl_ = slice(hh * (S // 2), (hh + 1) * (S // 2))
                    nc.sync.dma_start(
                        out=x8_sb[:, t, sl_],
                        in_=x8_d[t * 128:(t + 1) * 128, sl_])
            x8sl_sb = data.tile([128, NT, SL], f8)
            nc.sync.dma_start(
                out=x8sl_sb[:], in_=x8sl_d.rearrange("(t p) s -> p t s", p=128))
            xsl_sb = data.tile([128, NT, SL], f32)
            nc.sync.dma_start(
                out=xsl_sb[:], in_=xsl_d.rearrange("(t p) s -> p t s", p=128))

            wq_bf = consts.tile([128, NT, C], bf16)
            wk_bf = consts.tile([128, NT, C], bf16)
            wv_bf = consts.tile([128, NT, C], bf16)
            pw8_sb = consts.tile([128, NT, C], f8)
            for dst, srcd in ((wq_bf, wq_d), (wk_bf, wk_d), (wv_bf, wv_d),
                              (pw8_sb, pw8_d)):
                nc.sync.dma_start(
                    out=dst[:], in_=srcd.rearrange("(t p) c -> p t c", p=128))
            bq_sb = consts.tile([128, NT, 1], f32)
            bk_sb = consts.tile([128, NT, 1], f32)
            pb_sb = consts.tile([128, NT, 1], f32)
            gnw_sb = consts.tile([128, NT, 1], f32)
            gnb_sb = consts.tile([128, NT, 1], f32)
            for dst, srcd in ((bq_sb, bq_d), (bk_sb, bk_d), (pb_sb, pb_d),
                              (gnw_sb, gnw_d), (gnb_sb, gnb_d)):
                nc.sync.dma_start(
                    out=dst[:], in_=srcd.rearrange("(t p) c -> p t c", p=128))
            g8_sb = consts.tile([128, 16], f32)
            nc.sync.dma_start(out=g8_sb[:], in_=g8_d[:, :])
            g8t_sb = consts.tile([16, 128], f32)
            nc.sync.dma_start(out=g8t_sb[:], in_=g8t_d[:, :])

            ones97 = consts.tile([97, 32], bf16)
            nc.vector.memset(ones97[:], 1.0)
            eps_sb = consts.tile([16, 1], f32)
            nc.vector.memset(eps_sb[:], EPS)
            lnup_sb = consts.tile([128, 1], f32)
            nc.vector.memset(lnup_sb[:], float(math.log(ATT_UP)))

            # ---------- GroupNorm stats -> per-channel affine A, B ----------
            a_sb = data.tile([128, NT, 1], f32)
            b_sb = data.tile([128, NT, 1], f32)
            b_bf = data.tile([128, NT, 1], bf16)

            from contextlib import ExitStack as _ES
            _head = _ES()
            ps_h = _head.enter_context(
                tc.tile_pool(name="ps_head", bufs=1, space="PSUM"))

            stats6 = data.tile([128, NT, 8, 6], f32)
            # all 16 bn_stats first (solid DVE stream), then both per-t
            # aggregation chains -- their PE/ACT round-trip waits overlap
            for t in range(NT):
                x_sg = x8_sb[:, t, :].rearrange("p (n f) -> p n f", f=512)
                for sg in range(8):
                    nc.vector.bn_stats(out=stats6[:, t, sg, :], in_=x_sg[:, sg, :])
            for t in range(NT):
                mv = work.tile([128, 2], f32, tag="mv", bufs=2)
                nc.vector.bn_aggr(out=mv[:], in_=stats6[:, t, :, :])
                st2 = work.tile([128, 2], f32, tag="st2", bufs=2)
                nc.vector.tensor_copy(out=st2[:, 0:1], in_=mv[:, 0:1])
                m2 = work.tile([128, 1], f32, tag="m2", bufs=2)
                nc.vector.tensor_mul(m2[:], mv[:, 0:1], mv[:, 0:1])
                nc.vector.tensor_add(st2[:, 1:2], mv[:, 1:2], m2[:])
                gstat_ps = ps_h.tile([16, 2], f32, tag="gmisc", bufs=1)
                nc.tensor.matmul(gstat_ps[:], g8_sb[:], st2[:], start=True,
                                 stop=True)
                gs = work.tile([16, 2], f32, tag="gs", bufs=2)
                nc.vector.tensor_copy(out=gs[:], in_=gstat_ps[:])
                gm2 = work.tile([16, 1], f32, tag="gm2", bufs=2)
                nc.vector.tensor_mul(gm2[:], gs[:, 0:1], gs[:, 0:1])
                gvar = work.tile([16, 1], f32, tag="gvar", bufs=2)
                nc.vector.tensor_tensor(out=gvar[:], in0=gs[:, 1:2],
                                        in1=gm2[:], op=Alu.subtract)
                mr = work.tile([16, 2], f32, tag="mr", bufs=2)
                nc.vector.tensor_copy(out=mr[:, 0:1], in_=gs[:, 0:1])
                # rsqrt(var+eps) = exp(-0.5*ln(var+eps)): stays in the
                # natural_log_exp table set (no ACT table swap vs Sqrt)
                lnv = work.tile([16, 1], f32, tag="gstd", bufs=2)
                nc.scalar.activation(out=lnv[:], in_=gvar[:], func=Act.Ln,
                                     bias=eps_sb[:], scale=1.0)
                nc.scalar.activation(out=mr[:, 1:2], in_=lnv[:], func=Act.Exp,
                                     scale=-0.5)
                bcast_ps = ps_h.tile([128, 2], f32, tag="gmisc", bufs=1)
                nc.tensor.matmul(bcast_ps[:], g8t_sb[:], mr[:], start=True,
                                 stop=True)
                nc.vector.tensor_mul(a_sb[:, t, :], bcast_ps[:, 1:2],
                                     gnw_sb[:, t, :])
                tmp = work.tile([128, 1], f32, tag="tmpB", bufs=2)
                nc.vector.tensor_mul(tmp[:], bcast_ps[:, 0:1], a_sb[:, t, :])
                nc.vector.tensor_tensor(out=b_sb[:, t, :], in0=gnb_sb[:, t, :],
                                        in1=tmp[:], op=Alu.subtract)
                nc.vector.tensor_copy(out=b_bf[:, t, :], in_=b_sb[:, t, :])

            # ---------- weight prep: w8 = w_bf * A (per input channel) ------
            wq8 = data.tile([128, NT, C], f8)
            wk8 = data.tile([128, NT, C], f8)
            wv8 = data.tile([128, NT, C], f8)
            for t in range(NT):
                for wsrc, wdst in ((wk_bf, wk8), (wq_bf, wq8), (wv_bf, wv8)):
                    nc.vector.tensor_scalar(
                        out=wdst[:, t, :], in0=wsrc[:, t, :],
                        scalar1=a_sb[:, t, :], scalar2=None, op0=Alu.mult)

            # ---------- bias folds (runtime GN shift B) ----------
            # K/Q bias' = host_bias + w_raw @ B ; V's B-term goes to proj bias
            kb2 = data.tile([128, NT, 1], f32)
            qb2 = data.tile([128, NT, 1], f32)
            kb2d = data.tile([128, NT, 1], f32)
            bias4_ps = ps_h.tile([128, 2, 4], f32, tag="gmisc", bufs=1)
            bias_ps = bias4_ps[:, :, 0:3]
            for g in range(NT):
                osl = slice(g * 128, (g + 1) * 128)
                for ci in range(NT):
                    nc.tensor.matmul(bias_ps[:, g, 0:1], wk_bf[:, ci, osl],
                                     b_bf[:, ci, :], start=(ci == 0),
                                     stop=(ci == 1))
                    nc.tensor.matmul(bias_ps[:, g, 1:2], wq_bf[:, ci, osl],
                                     b_bf[:, ci, :], start=(ci == 0),
                                     stop=(ci == 1))
                    nc.tensor.matmul(bias_ps[:, g, 2:3], wv_bf[:, ci, osl],
                                     b_bf[:, ci, :], start=(ci == 0),
                                     stop=(ci == 1))
            wvB8 = data.tile([128, NT, 1], f8)
            for g in range(NT):
                # host bq/bk are pre-scaled by u*WQK_HOST, matching bias_ps
                nc.vector.tensor_scalar(
                    out=kb2[:, g, :], in0=bias_ps[:, g, 0:1],
                    scalar1=bk_sb[:, g, :], scalar2=None, op0=Alu.add)
                nc.vector.tensor_scalar(
                    out=qb2[:, g, :], in0=bias_ps[:, g, 1:2],
                    scalar1=bq_sb[:, g, :], scalar2=None, op0=Alu.add)
                nc.vector.tensor_scalar(
                    out=kb2d[:, g, :], in0=bias_ps[:, g, 0:1],
                    scalar1=bk_sb[:, g, :], scalar2=1.0 / WQK_HOST,
                    op0=Alu.add, op1=Alu.mult)
                # wv@B in true v units (compensate host wv scale)
                nc.vector.tensor_scalar(
                    out=wvB8[:, g, :], in0=bias_ps[:, g, 2:3],
                    scalar1=1.0 / WVP_HOST, scalar2=None, op0=Alu.mult)
            # pbp = pb_host + pw @ wvB   (proj bias pickup of GN shift via V)
            pvb_ps = bias4_ps[:, :, 3:4]
            for oi in range(NT):
                for dh in range(NT):
                    nc.tensor.matmul(
                        pvb_ps[:, oi, :], pw8_sb[:, dh, oi * 128:(oi + 1) * 128],
                        wvB8[:, dh, :], start=(dh == 0), stop=(dh == 1))
            pbp = data.tile([128, NT, 1], f32)
            for oi in range(NT):
                nc.vector.tensor_scalar(
                    out=pbp[:, oi, :], in0=pvb_ps[:, oi, :],
                    scalar1=1.0 / WVP_HOST, scalar2=pb_sb[:, oi, :],
                    op0=Alu.mult, op1=Alu.add)
            # xpb = xsl + pbp  (residual + proj bias, consumed by final stt)
            # on the otherwise-idle Pool engine (SBUF-only op)
            xpb = data.tile([128, NT, SL], f32)
            for oi in range(NT):
                nc.vector.tensor_scalar(
                    out=xpb[:, oi, :], in0=xsl_sb[:, oi, :],
                    scalar1=pbp[:, oi, :], scalar2=None, op0=Alu.add)

            # ---------- K / Q / V (fp8 DoubleRow) ----------
            kmat = data.tile([128, NT, S], bf16)
            qmat = data.tile([128, NT, SL], bf16)
            # head stride padded to 36 (4B-aligned fp8 weight rows for DR
            # ldweights); col 32 = ones (denominator), cols 33-35 = 0 pad
            vt8 = data.tile([128, PAIRS, 2, NH, 36], bf16)
            nc.gpsimd.memset(vt8[:, :, :, :, 32:33], 1.0)
            nc.gpsimd.memset(vt8[:, :, :, :, 33:36], 0.0)

            lnden = data.tile([97, NT, 2, SL], bf16)
            attn8 = data.tile([128, NT, SL], f8)
            osl_sb = data.tile([128, NT, SL], f32)
            prj_s = ATT_UP * WVP_HOST
            ln_attup = float(math.log(ATT_UP))

            def kq_piece(g, kind, j, kq_pool, kq_tag, kq_bufs):
                osl = slice(g * 128, (g + 1) * 128)
                if kind == "k":
                    ps_k = kq_pool.tile([128, 512], f32, tag=kq_tag,
                                        bufs=kq_bufs, name="ps_k")
                    for ci in range(NT):
                        nc.tensor.matmul(ps_k[:], wk8[:, ci, osl],
                                         x8_sb[:, ci, j * 512:(j + 1) * 512],
                                         start=(ci == 0), stop=(ci == NT - 1))
                    if j % 2 == 0:
                        nc.vector.tensor_scalar(
                            out=kmat[:, g, j * 512:(j + 1) * 512], in0=ps_k[:],
                            scalar1=kb2[:, g, :], scalar2=1.0 / WQK_HOST,
                            op0=Alu.add, op1=Alu.mult)
                    else:
                        nc.scalar.activation(
                            out=kmat[:, g, j * 512:(j + 1) * 512], in_=ps_k[:],
                            func=Act.Identity, bias=kb2d[:, g, :],
                            scale=1.0 / WQK_HOST)
                else:
                    ps_q = kq_pool.tile([128, 512], f32, tag=kq_tag,
                                        bufs=kq_bufs, name="ps_q")
                    for ci in range(NT):
                        nc.tensor.matmul(ps_q[:], wq8[:, ci, osl],
                                         x8sl_sb[:, ci,
                                                 j * 512:(j + 1) * 512],
                                         start=(ci == 0), stop=(ci == NT - 1))
                    nc.vector.tensor_scalar(
                        out=qmat[:, g, j * 512:(j + 1) * 512], in0=ps_q[:],
                        scalar1=qb2[:, g, :], scalar2=1.0 / WQK_HOST,
                        op0=Alu.add, op1=Alu.mult)

            def v_piece(p, v_pool, v_tag, v_bufs):
                ps_v = v_pool.tile([128, 2, 256], f32, tag=v_tag,
                                   bufs=v_bufs, name="ps_v")
                for par in range(2):
                    tch = 2 * p + par
                    # plain fp8 2-pass accumulate: DR's ldweights penalty
                    # makes it slower than two N=256 passes here
                    for ci in range(NT):
                        nc.tensor.matmul(
                            ps_v[:, par, :],
                            x8_sb[:, ci, tch * 128:(tch + 1) * 128],
                            wv8[:, ci, :], start=(ci == 0),
                            stop=(ci == NT - 1))
                src = ps_v[:].rearrange("p two (h d) -> p two h d", d=32)
                if p % 2 == 0:
                    nc.vector.tensor_scalar(
                        out=vt8[:, p, :, :, 0:32], in0=src,
                        scalar1=1.0 / WVP_HOST, scalar2=None, op0=Alu.mult)
                else:
                    nc.scalar.activation(
                        out=vt8[:, p, :, :, 0:32], in_=src, func=Act.Copy,
                        scale=1.0 / WVP_HOST)

            for g_ in range(NT):
                for j in range(S // 512):
                    kq_piece(g_, "k", j, ps_h, "ps_k", 4)
                for j in range(SL // 512):
                    kq_piece(g_, "q", j, ps_h, "ps_k", 4)
            for p in range(PAIRS):
                v_piece(p, ps_h, "ps_v", 3)

            _head.close()

            if True:

                def do_att(g, half, ps_att):
                    h0 = g * 4
                    s0_ = half * 512
                    qs = qmat[:, g, s0_:s0_ + 512]
                    av0 = ps_att.tile([128, 512], f32, tag="ps_av", bufs=2)
                    av1 = ps_att.tile([128, 512], f32, tag="ps_av", bufs=2)
                    ex_q = {}
                    for tt in range(TCH + LA):
                        if tt < TCH:
                            tch = tt
                            ks = slice(tch * 128, (tch + 1) * 128)
                            ps_a = ps_att.tile([128, 1024], f32,
                                               tag="ps_sc", bufs=3)
                            ps_b = ps_att.tile([128, 1024], f32,
                                               tag="ps_sc", bufs=3)
                            # bf16 scores, 4-way row packed
                            for r, (pst, col) in enumerate(
                                ((ps_a, 0), (ps_a, 512), (ps_b, 0),
                                 (ps_b, 512))
                            ):
                                rb = r * 32
                                nc.tensor.matmul(
                                    pst[:, col:col + 512],
                                    kmat[rb:rb + 32, g, ks],
                                    qs[rb:rb + 32, :],
                                    start=True, stop=True,
                                    tile_position=(rb, 0),
                                )
                            exA = work.tile([128, 1024], bf16,
                                            tag="exA", bufs=6)
                            exB = work.tile([128, 1024], bf16,
                                            tag="exB", bufs=6)

                            # heads h0,h0+1 -> ACT exact exp
                            nc.scalar.activation(
                                out=exA[:], in_=ps_a[:],
                                func=Act.Exp, scale=float(1.0 / GAMMA))
                            # heads h0+2,h0+3 -> DVE cubic (moving any exB
                            # tile to ACT serializes behind exA there and
                            # delays that chunk's AV: measured +7us)
                            nc.vector._custom_dve(
                                expc, out=exB[:], in0=ps_b[:],
                                s0=C0_DVE, s1=C1_DVE)
                            ex_q[tch] = (exA, exB)
                        if tt >= LA:
                            tch = tt - LA
                            eA, eB = ex_q.pop(tch)
                            first, last = tch == 0, tch == TCH - 1
                            for av, col, ex, xcol in (
                                (av0, 0, eA, 0),      # h0
                                (av1, 0, eA, 512),    # h0+1
                                (av0, 64, eB, 0),     # h0+2
                                (av1, 64, eB, 512),   # h0+3
                            ):
                                nc.tensor.matmul(
                                    av[col:col + 33, :],
                                    vt8[:, tch // 2, tch % 2,
                                        h0 + (0 if col == 0 else 2)
                                        + (0 if av is av0 else 1), 0:33],
                                    ex[:, xcol:xcol + 512],
                                    start=first, stop=last,
                                    skip_group_check=True,
                                )
                    # softmax tail: ln of the ridden denominators, PE
                    # broadcast, rec = exp(-ln den + ln ATT_UP), fused
                    # psum-evac * rec -> attn8 (one ACT table set)
                    for avi, av in ((0, av0), (1, av1)):
                        for p_ in (32, 96):
                            nc.scalar.activation(
                                out=lnden[p_:p_ + 1, g, avi, s0_:s0_ + 512],
                                in_=av[p_:p_ + 1, :], func=Act.Ln)
                    rec_ps = ps_att.tile([128, 1024], f32, tag="ps_sc",
                                         bufs=3)
                    for r in range(4):
                        p_ = 32 if r < 2 else 96
                        avi = r % 2
                        nc.tensor.matmul(
                            rec_ps[r * 32:(r + 1) * 32, 0:512],
                            ones97[p_:p_ + 1, :],
                            lnden[p_:p_ + 1, g, avi, s0_:s0_ + 512],
                            start=True, stop=True,
                            tile_position=(p_, r * 32),
                            skip_group_check=True,
                        )
                    rec_bf = work.tile([128, 512], bf16, tag="rec", bufs=2)
                    nc.scalar.activation(out=rec_bf[:], in_=rec_ps[:, 0:512],
                                         func=Act.Exp, scale=-1.0,
                                         bias=lnup_sb[:])

                    for av, col, ob in (
                        (av0, 0, 0), (av1, 0, 32),
                        (av0, 64, 64), (av1, 64, 96),
                    ):
                        nc.vector.tensor_mul(
                            attn8[ob:ob + 32, g, s0_:s0_ + 512],
                            av[col:col + 32, :], rec_bf[ob:ob + 32, :])

                def do_proj(j, ps_att):
                    for oi in range(NT):
                        ps_p = ps_att.tile([128, 512], f32, tag="ps_av",
                                           bufs=2)
                        for ci in range(NT):
                            nc.tensor.matmul(
                                ps_p[:],
                                pw8_sb[:, ci, oi * 128:(oi + 1) * 128],
                                attn8[:, ci, j * 512:(j + 1) * 512],
                                start=(ci == 0), stop=(ci == NT - 1))
                        nc.vector.scalar_tensor_tensor(
                            out=osl_sb[:, oi, j * 512:(j + 1) * 512],
                            in0=ps_p[:], scalar=float(1.0 / prj_s),
                            in1=xpb[:, oi, j * 512:(j + 1) * 512],
                            op0=Alu.mult, op1=Alu.add)
                        nc.sync.dma_start(
                            out=out_d[oi * 128:(oi + 1) * 128,
                                      j * 512:(j + 1) * 512],
                            in_=osl_sb[:, oi, j * 512:(j + 1) * 512])

                with tc.tile_pool(name="ps_att", bufs=1,
                                  space="PSUM") as ps_att:
                    do_att(0, 0, ps_att)
                    do_att(1, 0, ps_att)
                    do_proj(0, ps_att)
                    do_att(0, 1, ps_att)
                    do_att(1, 1, ps_att)
                    do_proj(1, ps_att)


    nc.compile()
    return nc


def get_program():
    global _PROGRAM
    if _PROGRAM is None:
        _PROGRAM = _build_program()
    return _PROGRAM


def make_in_maps(x, gn_w, gn_b, qkv_w, qkv_b, proj_w, proj_b):
    """Host-side prep: cast/scale weights, shard x."""
    x = np.asarray(x, dtype=np.float32)
    xf = x.reshape(B, C, S)

    qkv_w = np.asarray(qkv_w, dtype=np.float32)
    qkv_b = np.asarray(qkv_b, dtype=np.float32)
    pw = np.asarray(proj_w, dtype=np.float32)

    u = math.sqrt(GAMMA / math.sqrt(HD))  # split of score prescale to q and k
    wq = (qkv_w[0:C] * (u * WQK_HOST)).T.astype(BF16)   # (c, o)
    wk = (qkv_w[C:2 * C] * (u * WQK_HOST)).T.astype(BF16)
    wv = (qkv_w[2 * C:3 * C] * WVP_HOST).T.astype(BF16)
    pw8 = (pw * WVP_HOST).T.astype(F8)
    # pre-scaled to match bias_ps units (w_bf @ B); descaled at K/Q evac
    bq = (qkv_b[0:C] * (u * WQK_HOST)).reshape(C, 1).astype(np.float32)
    bk = (qkv_b[C:2 * C] * (u * WQK_HOST)).reshape(C, 1).astype(np.float32)
    # V bias: softmax weights sum to 1 -> fold proj_w @ bv into proj bias
    pb = (np.asarray(proj_b, dtype=np.float32)
          + pw @ qkv_b[2 * C:3 * C]).reshape(C, 1)
    gnw = np.asarray(gn_w, dtype=np.float32).reshape(C, 1)
    gnb = np.asarray(gn_b, dtype=np.float32).reshape(C, 1)

    g8 = np.zeros((128, 16), np.float32)
    g8t = np.zeros((16, 128), np.float32)
    for p in range(128):
        g8[p, p // CPG] = 1.0 / CPG
        g8t[p // CPG, p] = 1.0

    common = dict(wq_t=wq, wk_t=wk, wv_t=wv, pw8_t=pw8, bq=bq, bk=bk, pb=pb,
                  gnw=gnw, gnb=gnb, g8=g8, g8t=g8t)
    in_maps = []
    for core in range(NCORES):
        bi, sl = core // NSL, core % NSL
        m = dict(common)
        xb = xf[bi]
        m["x8"] = np.ascontiguousarray(xb).astype(F8)
        m["x8sl"] = np.ascontiguousarray(
            xb[:, sl * SL:(sl + 1) * SL]).astype(F8)
        m["xsl"] = np.ascontiguousarray(xb[:, sl * SL:(sl + 1) * SL])
        in_maps.append(m)
    return in_maps


def kernel(x, gn_w, gn_b, qkv_w, qkv_b, proj_w, proj_b):
    global LAST_RESULTS
    from concourse.bass_utils import run_bass_kernel_spmd

    nc = get_program()
    in_maps = make_in_maps(x, gn_w, gn_b, qkv_w, qkv_b, proj_w, proj_b)
    res = run_bass_kernel_spmd(nc, in_maps, list(range(NCORES)))
    LAST_RESULTS = res
    out = np.empty((B, C, S), np.float32)
    for core in range(NCORES):
        bi, sl = core // NSL, core % NSL
        out[bi][:, sl * SL:(sl + 1) * SL] = res.results[core]["out_sl"]
    return out.reshape(B, C, H, W).astype(np.float32)



# revision 55
# speedup vs baseline: 1.0065x; 1.0065x over previous
"""Trainium2 Bass kernel for nn_AttentionBlock (GroupNorm -> MHA(8 heads,
s=4096) -> proj -> residual).

Sharding: 8 cores = 2 batches x 4 query-token slices (1024 tokens each).
Each core: GroupNorm stats + full K/V for its batch, Q for its slice,
streaming softmax attention over its slice, projection + residual.

Design notes (vs the earlier bf16 baseline):
- x uploads as fp8e4 (1MB) plus an f32 residual slice; GroupNorm's
  per-channel affine (A, B) is folded into the QKV weights at runtime, so xn
  is never materialized (saves a full DVE pass).  QKV weights upload bf16 and
  are runtime-scaled by A into fp8.
- K/Q/V/proj matmuls are plain fp8 2-pass accumulate (measured faster than
  fp8 DoubleRow here: DR's ldweights penalty eats the pair win at these
  free dims).  Scores are bf16 with 4-way PE row packing; AV is bf16
  33-wide (V + ones denominator column) with 2-way PE column packing.
- exp split: ACT hardware Exp for the (h0,h0+1) tiles; DVE 3-uop cubic
  custom op (1 + C1 s + s^2 + C0 s^3 ~ exp(s/g), scores prescaled by
  g=1/sqrt(2) in the host weight scale) for all (h0+2,h0+3) tiles.
  Rates: ACT ~1.11us, DVE ~1.22us per [128,1024] tile (DVE is 1x from
  f32 PSUM; TRN2 matmul cannot write 16-bit PSUM, so no 2X_1PORT).
- softmax denominator rides the AV accumulation via the ones column.
  rec = exp(-ln(den) + ln(ATT_UP)): Ln on the den rows, tiny PE broadcast
  matmuls, then ACT Exp.  Together with GN's rsqrt = exp(-0.5 ln(var+eps))
  every ACT function used lives in the natural_log_exp_and_others table
  set (enforced by trimming Exp/Ln from the other sets pre-compile), so
  exactly one ACT_TABLE_LOAD is emitted (vs ~10 reloads at 2.7us each
  when Reciprocal/Sqrt alternate with Exp).
- AV evacuation is fused with normalization: attn8 = av(psum) * rec in 4
  [32,512] DVE tensor_tensors (no out_un8 staging).
- dtype scale management: wq/wk host-scaled by 64, wv/pw by 16 to keep fp8
  weights in the e4m3 normal range; compensated at psum evacuation.
  attn8 is upscaled by 64 (folded into the denominator) for the same reason.

Scheduling notes (measured, not guessed):
- steady state is three-way engine-balanced: PE ~87%, ACT ~82%, DVE ~78%;
  per chunk 2 exp tiles (ACT+DVE in parallel) bound the cadence at
  ~1.2us with PE close behind.
- interleaving head-phase K/Q/V pieces or deferred softmax tails into the
  attention chunk loop via the ps_sc psum ring REGRESSES (ring-depth
  collapse serializes the score pipeline); keep the head serial and the
  tails inline.
- PSUM is fully committed: 3x [128,1024] f32 score tiles + 2 AV banks;
  rec_ps reuses a ps_sc slot at the tail.

Self-contained: hardcodes shapes (x: (2,256,64,64) f32).
"""

import math
import sys

import numpy as np

sys.path.insert(0, "/opt/trn_rl_repo")

import ml_dtypes  # noqa: E402

F8 = ml_dtypes.float8_e4m3
BF16 = ml_dtypes.bfloat16

# ---- problem constants ----
B, C, H, W = 2, 256, 64, 64
S = H * W            # 4096 tokens
NH, HD = 8, 32       # heads, head dim
GROUPS = 32
CPG = C // GROUPS
EPS = 1e-5
NCORES = 8
NSL = 4              # token slices per batch
SL = S // NSL        # 1024 query tokens per core
NT = C // 128        # 2 channel tiles
TCH = S // 128       # 32 key chunks
PAIRS = TCH // 2     # 16 key chunk pairs

# score prescale gamma: psum scores = GAMMA * s_true; exp(s') with the DVE
# cubic 1 + C1 s' + s'^2 + C0 s'^3 needs quad coeff 1 -> GAMMA = 1/sqrt(2)
GAMMA = 2.0 ** -0.5
C1_DVE = 1.414203
C0_DVE = 0.473414
WQK_HOST = 64.0      # fp8 range scale on wq/wk (compensated at K/Q evac)
WVP_HOST = 16.0      # fp8 range scale on wv/pw (compensated at V evac / den)
ATT_UP = 64.0        # attn8 upscale (folded into den) to stay in fp8 normal
LA = 2               # exp->AV software pipeline lookahead (chunks)

_PROGRAM = None
LAST_RESULTS = None
_EXPC_OP = None


def _register_expc():
    """ANT_EXPC3: 1 + s*(C1 + s*(1 + s*C0)) = 1 + C1 s + s^2 + C0 s^3."""
    global _EXPC_OP
    if _EXPC_OP is not None:
        return _EXPC_OP
    from concourse import dve_ops
    from concourse.dve_spec import Spec, Src0, C0, C1, One, lower
    from concourse.dve_uop import DveOpSpec
    from concourse.dve_ops import DveOp

    for op in dve_ops.OPS:
        if op.name == "ANT_EXPC3":
            _EXPC_OP = op
            return op

    def _ref(in0, in1, s0, s1, imm2):
        t1 = 1.0 + in0 * s0
        t2 = s1 + in0 * t1
        t3 = 1.0 + in0 * t2
        return t3.astype(np.float32)

    t1 = One + Src0 * C0
    t2 = C1 + Src0 * t1
    t3 = One + Src0 * t2
    spec = Spec(body=t3, reference=_ref)
    shas = {
        ver: DveOpSpec(
            name="ANT_EXPC3", opcode=0, uops=lower(spec, ver=ver), rd1_en=False
        ).sha(ver)
        for ver in ("v3", "v4")
    }
    op = DveOp("ANT_EXPC3", spec, subdim=False, uops_sha=shas)
    dve_ops.OPS.append(op)
    dve_ops._SUB_OPCODE_FOR_NAME[op.name] = (
        dve_ops._CUSTOM_DVE_ROW_BASE + len(dve_ops.OPS) - 1
    )
    dve_ops.CUSTOM_DVE_SPECS[op.name] = spec
    _EXPC_OP = op
    return op


def _build_program():
    import concourse.bass as bass  # noqa: F401
    import concourse.tile as tile
    from concourse import bacc, mybir

    f32 = mybir.dt.float32
    bf16 = mybir.dt.bfloat16
    f8 = mybir.dt.float8e4
    Alu = mybir.AluOpType
    Act = mybir.ActivationFunctionType
    DR = mybir.MatmulPerfMode.DoubleRow
    DP = mybir.MatmulPerfMode.DoublePixel

    expc = _register_expc()

    # All ACT functions this kernel uses (Exp, Ln, Copy, Identity) live in
    # the natural_log_exp_and_others table set.  The load-insertion pass
    # would otherwise pick exp_and_others for Exp and natural_log for Ln,
    # reloading ACT tables (~2.7us each) at every softmax tail.  Trim Exp/Ln
    # from the other sets (membership only -- set order/ids stay valid) so
    # one set covers everything and exactly one load is emitted.
    from concourse import hw_specs

    nc = bacc.Bacc(
        "TRN2",
        target_bir_lowering=False,
        debug=False,
        enable_asserts=False,
        num_devices=NCORES,
    )

    _keep = "natural_log_exp_and_others"
    _tabs = hw_specs.get_activation_tables(nc.m.arch)
    assert _keep in _tabs and Act.Exp in _tabs[_keep] and Act.Ln in _tabs[_keep]
    for _name, _fns in _tabs.items():
        if _name != _keep:
            _fns.discard(Act.Exp)
            _fns.discard(Act.Ln)

    # ---- DRAM I/O ----
    x8_d = nc.dram_tensor("x8", [C, S], f8, kind="ExternalInput").ap()
    x8sl_d = nc.dram_tensor("x8sl", [C, SL], f8, kind="ExternalInput").ap()
    xsl_d = nc.dram_tensor("xsl", [C, SL], f32, kind="ExternalInput").ap()
    wq_d = nc.dram_tensor("wq_t", [C, C], bf16, kind="ExternalInput").ap()
    wk_d = nc.dram_tensor("wk_t", [C, C], bf16, kind="ExternalInput").ap()
    wv_d = nc.dram_tensor("wv_t", [C, C], bf16, kind="ExternalInput").ap()
    pw8_d = nc.dram_tensor("pw8_t", [C, C], f8, kind="ExternalInput").ap()
    bq_d = nc.dram_tensor("bq", [C, 1], f32, kind="ExternalInput").ap()
    bk_d = nc.dram_tensor("bk", [C, 1], f32, kind="ExternalInput").ap()
    pb_d = nc.dram_tensor("pb", [C, 1], f32, kind="ExternalInput").ap()
    gnw_d = nc.dram_tensor("gnw", [C, 1], f32, kind="ExternalInput").ap()
    gnb_d = nc.dram_tensor("gnb", [C, 1], f32, kind="ExternalInput").ap()
    g8_d = nc.dram_tensor("g8", [128, 16], f32, kind="ExternalInput").ap()
    g8t_d = nc.dram_tensor("g8t", [16, 128], f32, kind="ExternalInput").ap()
    out_d = nc.dram_tensor("out_sl", [C, SL], f32, kind="ExternalOutput").ap()

    with tile.TileContext(nc) as tc:
        with tc.tile_pool(name="consts", bufs=1) as consts, \
             tc.tile_pool(name="data", bufs=1) as data, \
             tc.tile_pool(name="work", bufs=3) as work:

            # ---------- load inputs ----------
            x8_sb = data.tile([128, NT, S], f8)
            for t in range(NT):
                for hh in range(2):
                    sl_ = slice(hh * (S // 2), (hh + 1) * (S // 2))
                    nc.sync.dma_start(
                        out=x8_sb[:, t, sl_],
                        in_=x8_d[t * 128:(t + 1) * 128, sl_])
            x8sl_sb = data.tile([128, NT, SL], f8)
            nc.sync.dma_start(
                out=x8sl_sb[:], in_=x8sl_d.rearrange("(t p) s -> p t s", p=128))
            xsl_sb = data.tile([128, NT, SL], f32)
            nc.sync.dma_start(
                out=xsl_sb[:], in_=xsl_d.rearrange("(t p) s -> p t s", p=128))

            wq_bf = consts.tile([128, NT, C], bf16)
            wk_bf = consts.tile([128, NT, C], bf16)
            wv_bf = consts.tile([128, NT, C], bf16)
            pw8_sb = consts.tile([128, NT, C], f8)
            for dst, srcd in ((wq_bf, wq_d), (wk_bf, wk_d), (wv_bf, wv_d),
                              (pw8_sb, pw8_d)):
                nc.sync.dma_start(
                    out=dst[:], in_=srcd.rearrange("(t p) c -> p t c", p=128))
            bq_sb = consts.tile([128, NT, 1], f32)
            bk_sb = consts.tile([128, NT, 1], f32)
            pb_sb = consts.tile([128, NT, 1], f32)
            gnw_sb = consts.tile([128, NT, 1], f32)
            gnb_sb = consts.tile([128, NT, 1], f32)
            for dst, srcd in ((bq_sb, bq_d), (bk_sb, bk_d), (pb_sb, pb_d),
                              (gnw_sb, gnw_d), (gnb_sb, gnb_d)):
                nc.sync.dma_start(
                    out=dst[:], in_=srcd.rearrange("(t p) c -> p t c", p=128))
            g8_sb = consts.tile([128, 16], f32)
            nc.sync.dma_start(out=g8_sb[:], in_=g8_d[:, :])
            g8t_sb = consts.tile([16, 128], f32)
            nc.sync.dma_start(out=g8t_sb[:], in_=g8t_d[:, :])

            ones97 = consts.tile([97, 32], bf16)
            nc.vector.memset(ones97[:], 1.0)
            eps_sb = consts.tile([16, 1], f32)
            nc.vector.memset(eps_sb[:], EPS)
            lnup_sb = consts.tile([128, 1], f32)
            nc.vector.memset(lnup_sb[:], float(math.log(ATT_UP)))

            # ---------- GroupNorm stats -> per-channel affine A, B ----------
            a_sb = data.tile([128, NT, 1], f32)
            b_sb = data.tile([128, NT, 1], f32)
            b_bf = data.tile([128, NT, 1], bf16)

            from contextlib import ExitStack as _ES
            _head = _ES()
            ps_h = _head.enter_context(
                tc.tile_pool(name="ps_head", bufs=1, space="PSUM"))

            stats6 = data.tile([128, NT, 8, 6], f32)
            # all 16 bn_stats first (solid DVE stream), then both per-t
            # aggregation chains -- their PE/ACT round-trip waits overlap
            for t in range(NT):
                x_sg = x8_sb[:, t, :].rearrange("p (n f) -> p n f", f=512)
                for sg in range(8):
                    nc.vector.bn_stats(out=stats6[:, t, sg, :], in_=x_sg[:, sg, :])
            for t in range(NT):
                mv = work.tile([128, 2], f32, tag="mv", bufs=2)
                nc.vector.bn_aggr(out=mv[:], in_=stats6[:, t, :, :])
                st2 = work.tile([128, 2], f32, tag="st2", bufs=2)
                nc.vector.tensor_copy(out=st2[:, 0:1], in_=mv[:, 0:1])
                m2 = work.tile([128, 1], f32, tag="m2", bufs=2)
                nc.vector.tensor_mul(m2[:], mv[:, 0:1], mv[:, 0:1])
                nc.vector.tensor_add(st2[:, 1:2], mv[:, 1:2], m2[:])
                gstat_ps = ps_h.tile([16, 2], f32, tag="gmisc", bufs=1)
                nc.tensor.matmul(gstat_ps[:], g8_sb[:], st2[:], start=True,
                                 stop=True)
                gs = work.tile([16, 2], f32, tag="gs", bufs=2)
                nc.vector.tensor_copy(out=gs[:], in_=gstat_ps[:])
                gm2 = work.tile([16, 1], f32, tag="gm2", bufs=2)
                nc.vector.tensor_mul(gm2[:], gs[:, 0:1], gs[:, 0:1])
                gvar = work.tile([16, 1], f32, tag="gvar", bufs=2)
                nc.vector.tensor_tensor(out=gvar[:], in0=gs[:, 1:2],
                                        in1=gm2[:], op=Alu.subtract)
                mr = work.tile([16, 2], f32, tag="mr", bufs=2)
                nc.vector.tensor_copy(out=mr[:, 0:1], in_=gs[:, 0:1])
                # rsqrt(var+eps) = exp(-0.5*ln(var+eps)): stays in the
                # natural_log_exp table set (no ACT table swap vs Sqrt)
                lnv = work.tile([16, 1], f32, tag="gstd", bufs=2)
                nc.scalar.activation(out=lnv[:], in_=gvar[:], func=Act.Ln,
                                     bias=eps_sb[:], scale=1.0)
                nc.scalar.activation(out=mr[:, 1:2], in_=lnv[:], func=Act.Exp,
                                     scale=-0.5)
                bcast_ps = ps_h.tile([128, 2], f32, tag="gmisc", bufs=1)
                nc.tensor.matmul(bcast_ps[:], g8t_sb[:], mr[:], start=True,
                                 stop=True)
                nc.vector.tensor_mul(a_sb[:, t, :], bcast_ps[:, 1:2],
                                     gnw_sb[:, t, :])
                tmp = work.tile([128, 1], f32, tag="tmpB", bufs=2)
                nc.vector.tensor_mul(tmp[:], bcast_ps[:, 0:1], a_sb[:, t, :])
                nc.vector.tensor_tensor(out=b_sb[:, t, :], in0=gnb_sb[:, t, :],
                                        in1=tmp[:], op=Alu.subtract)
                nc.vector.tensor_copy(out=b_bf[:, t, :], in_=b_sb[:, t, :])

            # ---------- weight prep: w8 = w_bf * A (per input channel) ------
            wq8 = data.tile([128, NT, C], f8)
            wk8 = data.tile([128, NT, C], f8)
            wv8 = data.tile([128, NT, C], f8)
            for t in range(NT):
                for wsrc, wdst in ((wk_bf, wk8), (wq_bf, wq8), (wv_bf, wv8)):
                    nc.vector.tensor_scalar(
                        out=wdst[:, t, :], in0=wsrc[:, t, :],
                        scalar1=a_sb[:, t, :], scalar2=None, op0=Alu.mult)

            # ---------- bias folds (runtime GN shift B) ----------
            # K/Q bias' = host_bias + w_raw @ B ; V's B-term goes to proj bias
            kb2 = data.tile([128, NT, 1], f32)
            qb2 = data.tile([128, NT, 1], f32)
            kb2d = data.tile([128, NT, 1], f32)
            bias4_ps = ps_h.tile([128, 2, 4], f32, tag="gmisc", bufs=1)
            bias_ps = bias4_ps[:, :, 0:3]
            for g in range(NT):
                osl = slice(g * 128, (g + 1) * 128)
                for ci in range(NT):
                    nc.tensor.matmul(bias_ps[:, g, 0:1], wk_bf[:, ci, osl],
                                     b_bf[:, ci, :], start=(ci == 0),
                                     stop=(ci == 1))
                    nc.tensor.matmul(bias_ps[:, g, 1:2], wq_bf[:, ci, osl],
                                     b_bf[:, ci, :], start=(ci == 0),
                                     stop=(ci == 1))
                    nc.tensor.matmul(bias_ps[:, g, 2:3], wv_bf[:, ci, osl],
                                     b_bf[:, ci, :], start=(ci == 0),
                                     stop=(ci == 1))
            wvB8 = data.tile([128, NT, 1], f8)
            for g in range(NT):
                # host bq/bk are pre-scaled by u*WQK_HOST, matching bias_ps
                nc.vector.tensor_scalar(
                    out=kb2[:, g, :], in0=bias_ps[:, g, 0:1],
                    scalar1=bk_sb[:, g, :], scalar2=None, op0=Alu.add)
                nc.vector.tensor_scalar(
                    out=qb2[:, g, :], in0=bias_ps[:, g, 1:2],
                    scalar1=bq_sb[:, g, :], scalar2=None, op0=Alu.add)
                nc.vector.tensor_scalar(
                    out=kb2d[:, g, :], in0=bias_ps[:, g, 0:1],
                    scalar1=bk_sb[:, g, :], scalar2=1.0 / WQK_HOST,
                    op0=Alu.add, op1=Alu.mult)
                # wv@B in true v units (compensate host wv scale)
                nc.vector.tensor_scalar(
                    out=wvB8[:, g, :], in0=bias_ps[:, g, 2:3],
                    scalar1=1.0 / WVP_HOST, scalar2=None, op0=Alu.mult)
            # pbp = pb_host + pw @ wvB   (proj bias pickup of GN shift via V)
            pvb_ps = bias4_ps[:, :, 3:4]
            for oi in range(NT):
                for dh in range(NT):
                    nc.tensor.matmul(
                        pvb_ps[:, oi, :], pw8_sb[:, dh, oi * 128:(oi + 1) * 128],
                        wvB8[:, dh, :], start=(dh == 0), stop=(dh == 1))
            pbp = data.tile([128, NT, 1], f32)
            for oi in range(NT):
                nc.vector.tensor_scalar(
                    out=pbp[:, oi, :], in0=pvb_ps[:, oi, :],
                    scalar1=1.0 / WVP_HOST, scalar2=pb_sb[:, oi, :],
                    op0=Alu.mult, op1=Alu.add)
            # xpb = xsl + pbp  (residual + proj bias, consumed by final stt)
            # on the otherwise-idle Pool engine (SBUF-only op)
            xpb = data.tile([128, NT, SL], f32)
            for oi in range(NT):
                nc.vector.tensor_scalar(
                    out=xpb[:, oi, :], in0=xsl_sb[:, oi, :],
                    scalar1=pbp[:, oi, :], scalar2=None, op0=Alu.add)

            # ---------- K / Q / V (fp8 DoubleRow) ----------
            kmat = data.tile([128, NT, S], bf16)
            qmat = data.tile([128, NT, SL], bf16)
            # head stride padded to 36 (4B-aligned fp8 weight rows for DR
            # ldweights); col 32 = ones (denominator), cols 33-35 = 0 pad
            vt8 = data.tile([128, PAIRS, 2, NH, 36], bf16)
            nc.gpsimd.memset(vt8[:, :, :, :, 32:33], 1.0)
            nc.gpsimd.memset(vt8[:, :, :, :, 33:36], 0.0)

            lnden = data.tile([97, NT, 2, SL], bf16)
            attn8 = data.tile([128, NT, SL], f8)
            osl_sb = data.tile([128, NT, SL], f32)
            prj_s = ATT_UP * WVP_HOST
            ln_attup = float(math.log(ATT_UP))

            def kq_piece(g, kind, j, kq_pool, kq_tag, kq_bufs):
                osl = slice(g * 128, (g + 1) * 128)
                if kind == "k":
                    ps_k = kq_pool.tile([128, 512], f32, tag=kq_tag,
                                        bufs=kq_bufs, name="ps_k")
                    for ci in range(NT):
                        nc.tensor.matmul(ps_k[:], wk8[:, ci, osl],
                                         x8_sb[:, ci, j * 512:(j + 1) * 512],
                                         start=(ci == 0), stop=(ci == NT - 1))
                    if j % 2 == 0:
                        nc.vector.tensor_scalar(
                            out=kmat[:, g, j * 512:(j + 1) * 512], in0=ps_k[:],
                            scalar1=kb2[:, g, :], scalar2=1.0 / WQK_HOST,
                            op0=Alu.add, op1=Alu.mult)
                    else:
                        nc.scalar.activation(
                            out=kmat[:, g, j * 512:(j + 1) * 512], in_=ps_k[:],
                            func=Act.Identity, bias=kb2d[:, g, :],
                            scale=1.0 / WQK_HOST)
                else:
                    ps_q = kq_pool.tile([128, 512], f32, tag=kq_tag,
                                        bufs=kq_bufs, name="ps_q")
                    for ci in range(NT):
                        nc.tensor.matmul(ps_q[:], wq8[:, ci, osl],
                                         x8sl_sb[:, ci,
                                                 j * 512:(j + 1) * 512],
                                         start=(ci == 0), stop=(ci == NT - 1))
                    nc.vector.tensor_scalar(
                        out=qmat[:, g, j * 512:(j + 1) * 512], in0=ps_q[:],
                        scalar1=qb2[:, g, :], scalar2=1.0 / WQK_HOST,
                        op0=Alu.add, op1=Alu.mult)

            def v_piece(p, v_pool, v_tag, v_bufs):
                ps_v = v_pool.tile([128, 2, 256], f32, tag=v_tag,
                                   bufs=v_bufs, name="ps_v")
                for par in range(2):
                    tch = 2 * p + par
                    # plain fp8 2-pass accumulate: DR's ldweights penalty
                    # makes it slower than two N=256 passes here
                    for ci in range(NT):
                        nc.tensor.matmul(
                            ps_v[:, par, :],
                            x8_sb[:, ci, tch * 128:(tch + 1) * 128],
                            wv8[:, ci, :], start=(ci == 0),
                            stop=(ci == NT - 1))
                src = ps_v[:].rearrange("p two (h d) -> p two h d", d=32)
                if p % 2 == 0:
                    nc.vector.tensor_scalar(
                        out=vt8[:, p, :, :, 0:32], in0=src,
                        scalar1=1.0 / WVP_HOST, scalar2=None, op0=Alu.mult)
                else:
                    nc.scalar.activation(
                        out=vt8[:, p, :, :, 0:32], in_=src, func=Act.Copy,
                        scale=1.0 / WVP_HOST)

            for g_ in range(NT):
                for j in range(S // 512):
                    kq_piece(g_, "k", j, ps_h, "ps_k", 4)
                for j in range(SL // 512):
                    kq_piece(g_, "q", j, ps_h, "ps_k", 4)
            for p in range(PAIRS):
                v_piece(p, ps_h, "ps_v", 3)

            _head.close()

            if True:

                def do_att(g, half, ps_att):
                    h0 = g * 4
                    s0_ = half * 512
                    qs = qmat[:, g, s0_:s0_ + 512]
                    av0 = ps_att.tile([128, 512], f32, tag="ps_av", bufs=2)
                    av1 = ps_att.tile([128, 512], f32, tag="ps_av", bufs=2)
                    ex_q = {}
                    for tt in range(TCH + LA):
                        if tt < TCH:
                            tch = tt
                            ks = slice(tch * 128, (tch + 1) * 128)
                            ps_a = ps_att.tile([128, 1024], f32,
                                               tag="ps_sc", bufs=3)
                            ps_b = ps_att.tile([128, 1024], f32,
                                               tag="ps_sc", bufs=3)
                            # bf16 scores, 4-way row packed
                            for r, (pst, col) in enumerate(
                                ((ps_a, 0), (ps_a, 512), (ps_b, 0),
                                 (ps_b, 512))
                            ):
                                rb = r * 32
                                nc.tensor.matmul(
                                    pst[:, col:col + 512],
                                    kmat[rb:rb + 32, g, ks],
                                    qs[rb:rb + 32, :],
                                    start=True, stop=True,
                                    tile_position=(rb, 0),
                                )
                            exA = work.tile([128, 1024], bf16,
                                            tag="exA", bufs=6)
                            exB = work.tile([128, 1024], bf16,
                                            tag="exB", bufs=6)

                            # heads h0,h0+1 -> ACT exact exp
                            nc.scalar.activation(
                                out=exA[:], in_=ps_a[:],
                                func=Act.Exp, scale=float(1.0 / GAMMA))
                            # heads h0+2,h0+3 -> DVE cubic (moving any exB
                            # tile to ACT serializes behind exA there and
                            # delays that chunk's AV: measured +7us)
                            nc.vector._custom_dve(
                                expc, out=exB[:], in0=ps_b[:],
                                s0=C0_DVE, s1=C1_DVE)
                            ex_q[tch] = (exA, exB)
                        if tt >= LA:
                            tch = tt - LA
                            eA, eB = ex_q.pop(tch)
                            first, last = tch == 0, tch == TCH - 1
                            for av, col, ex, xcol in (
                                (av0, 0, eA, 0),      # h0
                                (av1, 0, eA, 512),    # h0+1
                                (av0, 64, eB, 0),     # h0+2
                                (av1, 64, eB, 512),   # h0+3
                            ):
                                nc.tensor.matmul(
                                    av[col:col + 33, :],
                                    vt8[:, tch // 2, tch % 2,
                                        h0 + (0 if col == 0 else 2)
                                        + (0 if av is av0 else 1), 0:33],
                                    ex[:, xcol:xcol + 512],
                                    start=first, stop=last,
                                    skip_group_check=True,
                                )
                    # softmax tail: ln of the ridden denominators, PE
                    # broadcast, rec = exp(-ln den + ln ATT_UP), fused
                    # psum-evac * rec -> attn8 (one ACT table set)
                    rec_ps = ps_att.tile([128, 1024], f32, tag="ps_sc",
                                         bufs=3)
                    for avi, av in ((0, av0), (1, av1)):
                        for p_ in (32, 96):
                            nc.scalar.activation(
                                out=lnden[p_:p_ + 1, g, avi, s0_:s0_ + 512],
                                in_=av[p_:p_ + 1, :], func=Act.Ln)
                    # PE keepalives: the tail idles the PE ~5us (ACT backlog
                    # delays the lns) -- past the ~3.4us HAM MID window, so
                    # the PE re-throttles to 1.2GHz for ~10us into the next
                    # do_att (measured 54.7us throttle-active).  Tiny matmuls
                    # chained on tail intermediates split the idle gap;
                    # outputs land in an unread corner of rec_ps.
                    nc.tensor.matmul(
                        rec_ps[64:96, 512:544],
                        lnden[32:33, g, 0, s0_:s0_ + 32],
                        ones97[32:33, 0:32],
                        start=True, stop=True, tile_position=(32, 64),
                        skip_group_check=True)
                    for r in range(4):
                        p_ = 32 if r < 2 else 96
                        avi = r % 2
                        nc.tensor.matmul(
                            rec_ps[r * 32:(r + 1) * 32, 0:512],
                            ones97[p_:p_ + 1, :],
                            lnden[p_:p_ + 1, g, avi, s0_:s0_ + 512],
                            start=True, stop=True,
                            tile_position=(p_, r * 32),
                            skip_group_check=True,
                        )
                    rec_bf = work.tile([128, 512], bf16, tag="rec", bufs=2)
                    nc.scalar.activation(out=rec_bf[:], in_=rec_ps[:, 0:512],
                                         func=Act.Exp, scale=-1.0,
                                         bias=lnup_sb[:])
                    nc.tensor.matmul(
                        rec_ps[0:32, 512:544],
                        rec_bf[0:1, 0:32],
                        ones97[0:1, 0:32],
                        start=True, stop=True, tile_position=(0, 0),
                        skip_group_check=True)

                    for av, col, ob in (
                        (av0, 0, 0), (av1, 0, 32),
                        (av0, 64, 64), (av1, 64, 96),
                    ):
                        nc.vector.tensor_mul(
                            attn8[ob:ob + 32, g, s0_:s0_ + 512],
                            av[col:col + 32, :], rec_bf[ob:ob + 32, :])

                def do_proj(j, ps_att):
                    for oi in range(NT):
                        ps_p = ps_att.tile([128, 512], f32, tag="ps_av",
                                           bufs=2)
                        for ci in range(NT):
                            nc.tensor.matmul(
                                ps_p[:],
                                pw8_sb[:, ci, oi * 128:(oi + 1) * 128],
                                attn8[:, ci, j * 512:(j + 1) * 512],
                                start=(ci == 0), stop=(ci == NT - 1))
                        nc.vector.scalar_tensor_tensor(
                            out=osl_sb[:, oi, j * 512:(j + 1) * 512],
                            in0=ps_p[:], scalar=float(1.0 / prj_s),
                            in1=xpb[:, oi, j * 512:(j + 1) * 512],
                            op0=Alu.mult, op1=Alu.add)
                        nc.sync.dma_start(
                            out=out_d[oi * 128:(oi + 1) * 128,
                                      j * 512:(j + 1) * 512],
                            in_=osl_sb[:, oi, j * 512:(j + 1) * 512])

                with tc.tile_pool(name="ps_att", bufs=1,
                                  space="PSUM") as ps_att:
                    do_att(0, 0, ps_att)
                    do_att(1, 0, ps_att)
                    do_proj(0, ps_att)
                    do_att(0, 1, ps_att)
                    do_att(1, 1, ps_att)
                    do_proj(1, ps_att)


    nc.compile()
    return nc


def get_program():
    global _PROGRAM
    if _PROGRAM is None:
        _PROGRAM = _build_program()
    return _PROGRAM


def make_in_maps(x, gn_w, gn_b, qkv_w, qkv_b, proj_w, proj_b):
    """Host-side prep: cast/scale weights, shard x."""
    x = np.asarray(x, dtype=np.float32)
    xf = x.reshape(B, C, S)

    qkv_w = np.asarray(qkv_w, dtype=np.float32)
    qkv_b = np.asarray(qkv_b, dtype=np.float32)
    pw = np.asarray(proj_w, dtype=np.float32)

    u = math.sqrt(GAMMA / math.sqrt(HD))  # split of score prescale to q and k
    wq = (qkv_w[0:C] * (u * WQK_HOST)).T.astype(BF16)   # (c, o)
    wk = (qkv_w[C:2 * C] * (u * WQK_HOST)).T.astype(BF16)
    wv = (qkv_w[2 * C:3 * C] * WVP_HOST).T.astype(BF16)
    pw8 = (pw * WVP_HOST).T.astype(F8)
    # pre-scaled to match bias_ps units (w_bf @ B); descaled at K/Q evac
    bq = (qkv_b[0:C] * (u * WQK_HOST)).reshape(C, 1).astype(np.float32)
    bk = (qkv_b[C:2 * C] * (u * WQK_HOST)).reshape(C, 1).astype(np.float32)
    # V bias: softmax weights sum to 1 -> fold proj_w @ bv into proj bias
    pb = (np.asarray(proj_b, dtype=np.float32)
          + pw @ qkv_b[2 * C:3 * C]).reshape(C, 1)
    gnw = np.asarray(gn_w, dtype=np.float32).reshape(C, 1)
    gnb = np.asarray(gn_b, dtype=np.float32).reshape(C, 1)

    g8 = np.zeros((128, 16), np.float32)
    g8t = np.zeros((16, 128), np.float32)
    for p in range(128):
        g8[p, p // CPG] = 1.0 / CPG
        g8t[p // CPG, p] = 1.0

    common = dict(wq_t=wq, wk_t=wk, wv_t=wv, pw8_t=pw8, bq=bq, bk=bk, pb=pb,
                  gnw=gnw, gnb=gnb, g8=g8, g8t=g8t)
    in_maps = []
    for core in range(NCORES):
        bi, sl = core // NSL, core % NSL
        m = dict(common)
        xb = xf[bi]
        m["x8"] = np.ascontiguousarray(xb).astype(F8)
        m["x8sl"] = np.ascontiguousarray(
            xb[:, sl * SL:(sl + 1) * SL]).astype(F8)
        m["xsl"] = np.ascontiguousarray(xb[:, sl * SL:(sl + 1) * SL])
        in_maps.append(m)
    return in_maps


def kernel(x, gn_w, gn_b, qkv_w, qkv_b, proj_w, proj_b):
    global LAST_RESULTS
    from concourse.bass_utils import run_bass_kernel_spmd

    nc = get_program()
    in_maps = make_in_maps(x, gn_w, gn_b, qkv_w, qkv_b, proj_w, proj_b)
    res = run_bass_kernel_spmd(nc, in_maps, list(range(NCORES)))
    LAST_RESULTS = res
    out = np.empty((B, C, S), np.float32)
    for core in range(NCORES):
        bi, sl = core // NSL, core % NSL
        out[bi][:, sl * SL:(sl + 1) * SL] = res.results[core]["out_sl"]
    return out.reshape(B, C, H, W).astype(np.float32)

